# revision 14
# baseline (speedup 1.0000x reference)
"""Trainium2 Bass kernel for nn_BitwiseHashing (v1 reconstruction).

fp16 d-major quad-packed stream; per d-chunk: 2 in-place quad adds +
2 folds on DVE, 8 b-major 64-col matmuls into PSUM [128,512]; bias via
ones-matmul; tanh + f32 y. PREF=4 software-pipelined emission.
"""

import numpy as np

import concourse.bacc as bacc
import concourse.mybir as mybir
from concourse import tile
from concourse.bass_utils import run_bass_kernel_spmd

L, B, D, K = 12, 8192, 1024, 64
NCORES = 8
BS = B // NCORES
P = 128
NDC = D // P
NLQ = 3
QW = 4 * BS
F32 = mybir.dt.float32
F16 = mybir.dt.float16

_nc_cache = None


def _build():
    global _nc_cache
    if _nc_cache is not None:
        return _nc_cache

    nc = bacc.Bacc("TRN2", target_bir_lowering=False, debug=False)
    x = nc.dram_tensor("x", [NLQ, D, QW], F16, kind="ExternalInput")
    wt = nc.dram_tensor("wt", [D, K], F16, kind="ExternalInput")
    bias = nc.dram_tensor("bias", [1, NDC * K], F16, kind="ExternalInput")
    y = nc.dram_tensor("y", [P, NDC * K], F32, kind="ExternalOutput")

    with tile.TileContext(nc) as tc:
        with (
            tc.tile_pool(name="const", bufs=1) as cpool,
            tc.tile_pool(name="xin", bufs=15) as xpool,
            tc.tile_pool(name="out", bufs=1) as opool,
            tc.tile_pool(name="po", bufs=1, space="PSUM") as ppool,
        ):
            bias_sb = cpool.tile([1, NDC * K], F16)
            nc.gpsimd.dma_start(out=bias_sb[:], in_=bias.ap())
            wt_sb = cpool.tile([P, NDC * K], F16)
            for dc in range(NDC):
                nc.gpsimd.dma_start(
                    out=wt_sb[:, dc * K:(dc + 1) * K],
                    in_=wt.ap()[dc * P:(dc + 1) * P, :],
                )
            ones_sb = cpool.tile([1, P], F16)
            nc.gpsimd.memset(ones_sb[:], 1.0)

            po = ppool.tile([P, NDC * K], F32)
            nc.tensor.matmul(
                po[:], lhsT=ones_sb[:], rhs=bias_sb[:], start=True, stop=False
            )

            xap = x.ap()

            def issue_loads(dc):
                d0 = dc * P
                ts = []
                for q in range(NLQ):
                    t = xpool.tile([P, QW], F16)
                    g = dc * NLQ + q
                    eng = nc.sync if g % 2 == 0 else nc.scalar
                    eng.dma_start(out=t[:], in_=xap[q, d0:d0 + P, :])
                    ts.append(t)
                return ts

            def reduce(ts):
                t0, t1, t2 = ts
                nc.vector.tensor_add(out=t0[:], in0=t0[:], in1=t1[:])
                nc.vector.tensor_add(out=t0[:], in0=t0[:], in1=t2[:])
                nc.vector.tensor_add(
                    out=t0[:, 0:2 * BS], in0=t0[:, 0:2 * BS],
                    in1=t0[:, 2 * BS:4 * BS],
                )
                nc.vector.tensor_add(
                    out=t0[:, 0:BS], in0=t0[:, 0:BS], in1=t0[:, BS:2 * BS]
                )
                return t0

            def project(dc, s):
                for blk in range(NDC):
                    nc.tensor.matmul(
                        po[:, blk * K:(blk + 1) * K],
                        lhsT=s[:, blk * P:(blk + 1) * P],
                        rhs=wt_sb[:, dc * K:(dc + 1) * K],
                        start=False,
                        stop=(dc == NDC - 1),
                    )

            PREF = 4
            tiles = {dc: issue_loads(dc) for dc in range(min(PREF, NDC))}
            for dc in range(NDC):
                s = reduce(tiles.pop(dc))
                if dc + PREF < NDC:
                    tiles[dc + PREF] = issue_loads(dc + PREF)
                project(dc, s)

            ysb = opool.tile([P, NDC * K], F32)
            nc.scalar.activation(
                ysb[:], po[:], mybir.ActivationFunctionType.Tanh
            )
            nc.sync.dma_start(out=y.ap()[:], in_=ysb[:])

    nc.compile()
    _nc_cache = nc
    return nc


def _ensure_ntff_hook():
    """Register the axon NTFF profile hook if the image's antenv lacks it."""
    import sys
    import types

    try:
        from antenv.axon_hooks import get_axon_ntff_profile_hook  # noqa: F401
        return
    except ImportError:
        pass
    import antenv

    mod = types.ModuleType("antenv.axon_hooks")
    mod._hook = None

    def set_axon_ntff_profile_hook(h):
        mod._hook = h

    def get_axon_ntff_profile_hook():
        return mod._hook

    mod.set_axon_ntff_profile_hook = set_axon_ntff_profile_hook
    mod.get_axon_ntff_profile_hook = get_axon_ntff_profile_hook
    sys.modules["antenv.axon_hooks"] = mod
    antenv.axon_hooks = mod
    try:
        from trn_agent_boot.trn_boot import _ntff_profile_via_ctypes

        mod._hook = _ntff_profile_via_ctypes("/opt/axon/libaxon_pjrt.so")
    except Exception:
        mod._hook = None


def _run(inputs, trace=False, **kwargs):
    x = np.asarray(inputs["x"], dtype=np.float32)
    W = np.asarray(inputs["W"], dtype=np.float32)
    b = np.asarray(inputs["b"], dtype=np.float32)
    wt = np.ascontiguousarray(W.T * np.float32(1.0 / L)).astype(np.float16)
    bias = np.tile(b.astype(np.float16), NDC).reshape(1, NDC * K)
    in_maps = []
    for c in range(NCORES):
        xs = x[:, c * BS:(c + 1) * BS, :]
        xq = xs.reshape(NLQ, 4, BS, D).transpose(0, 3, 1, 2)
        xq = np.ascontiguousarray(xq, dtype=np.float16).reshape(NLQ, D, QW)
        in_maps.append({"x": xq, "wt": wt, "bias": bias})
    if trace:
        _ensure_ntff_hook()
        import concourse.bass_utils as bu

        bu.upload_artifacts = lambda tmpdir: "local://skipped"
    nc = _build()
    res = run_bass_kernel_spmd(
        nc, in_maps, core_ids=list(range(NCORES)), trace=trace, **kwargs
    )
    ys = []
    for r in res.results:
        yr = r["y"].reshape(P, NDC, K).transpose(1, 0, 2).reshape(BS, K)
        ys.append(np.ascontiguousarray(yr))
    return np.concatenate(ys, axis=0), res


def kernel(**inputs):
    y, _ = _run(inputs)
    return y


# revision 15
# speedup vs baseline: 1.1407x; 1.1407x over previous
"""Trainium2 Bass kernel for nn_BitwiseHashing.

Computes out = tanh(mean_l(x) @ W.T + b) for x:[12,8192,1024] f32,
W:[64,1024], b:[64] -> out:[8192,64].

Strategy (data-parallel over 8 NeuronCores, memory-bound):
  - shard x along batch: 1024 batch cols per core; host casts to fp16
    (rel-err gate 2e-2 leaves ~25x margin) and pre-transposes to
    d-major l-quad-packed layout [3(lq), 1024(d), 4(i)*1024(b)]:
    24 MiB/core streamed at ~420 GB/s, 8 KiB descriptors, both HWDGE
    rings balanced, buffers deep enough that compute never
    back-pressures the stream.
  - per d-chunk dc0..dc6: DVE sums 3 quad tiles in-place and folds
    4096->2048->1024 (2.29/2.29/1.22/0.69 us), then 8 cheap 64-col
    matmuls (lhsT = folded sum, b-major) accumulate into PSUM
    [128(b), 512(dc-major k)]; bias pre-seeded via a C=1 ones-matmul.
  - tail: dc7's first quad loads FIRST and pre-folds early; its other
    8 l-slices arrive last as separate [128,1024] DMAs with one
    0.69 us running add each, pipelined behind the stream - after the
    final byte only one small add + 8 tiny matmuls + tanh + a 128 KiB
    fp16 y DMA remain. Host undoes the block-major y layout.
"""

import numpy as np

import concourse.bacc as bacc
import concourse.mybir as mybir
from concourse import tile
from concourse.bass_utils import run_bass_kernel_spmd

L, B, D, K = 12, 8192, 1024, 64
NCORES = 8
BS = B // NCORES      # 1024 batch columns per core
P = 128               # partitions
NDC = D // P          # 8 contraction chunks
NLQ = 3               # l-quads (12 layers = 3 quads of 4)
QW = 4 * BS           # 4096 cols per quad tile
F32 = mybir.dt.float32
F16 = mybir.dt.float16

_nc_cache = None


def _build():
    global _nc_cache
    if _nc_cache is not None:
        return _nc_cache

    nc = bacc.Bacc("TRN2", target_bir_lowering=False, debug=False)
    x = nc.dram_tensor("x", [NLQ, D, QW], F16, kind="ExternalInput")
    wt = nc.dram_tensor("wt", [P, NDC * K], F16, kind="ExternalInput")
    bias = nc.dram_tensor("bias", [1, NDC * K], F16, kind="ExternalInput")
    y = nc.dram_tensor("y", [P, NDC * K], F16, kind="ExternalOutput")

    LD = NDC - 1  # the d-chunk handled quad-first + slices-last

    with tile.TileContext(nc) as tc:
        with (
            tc.tile_pool(name="const", bufs=1) as cpool,
            tc.tile_pool(name="xlast", bufs=1) as lpool,
            tc.tile_pool(name="xin", bufs=15) as xpool,
            tc.tile_pool(name="xsl", bufs=8) as spool,
            tc.tile_pool(name="out", bufs=1) as opool,
            tc.tile_pool(name="po", bufs=1, space="PSUM") as ppool,
        ):
            # constants over SWDGE; both HWDGE rings carry only x + y
            bias_sb = cpool.tile([1, NDC * K], F16)
            nc.gpsimd.dma_start(out=bias_sb[:], in_=bias.ap())
            wt_sb = cpool.tile([P, NDC * K], F16)
            nc.gpsimd.dma_start(out=wt_sb[:], in_=wt.ap())
            ones_sb = cpool.tile([1, P], F16)
            nc.gpsimd.memset(ones_sb[:], 1.0)

            po = ppool.tile([P, NDC * K], F32)
            nc.tensor.matmul(
                po[:], lhsT=ones_sb[:], rhs=bias_sb[:], start=True, stop=False
            )

            xap = x.ap()
            gctr = [0]

            def ring():
                eng = nc.sync if gctr[0] % 2 == 0 else nc.scalar
                gctr[0] += 1
                return eng

            # ---- all loads upfront, in stream order ----
            d0L = LD * P
            tL = lpool.tile([P, QW], F16)            # dc7 quad 0, first
            ring().dma_start(out=tL[:], in_=xap[0, d0L:d0L + P, :])

            quads = {}
            for dc in range(NDC - 1):
                d0 = dc * P
                ts = []
                for q in range(NLQ):
                    t = xpool.tile([P, QW], F16)
                    ring().dma_start(out=t[:], in_=xap[q, d0:d0 + P, :])
                    ts.append(t)
                quads[dc] = ts

            slices = []                               # dc7 quads 1,2 as l-slices
            for q in (1, 2):
                for i in range(4):
                    t = spool.tile([P, BS], F16)
                    ring().dma_start(
                        out=t[:],
                        in_=xap[q, d0L:d0L + P, i * BS:(i + 1) * BS],
                    )
                    slices.append(t)

            # ---- compute ----
            def fold(t):
                nc.vector.tensor_add(
                    out=t[:, 0:2 * BS], in0=t[:, 0:2 * BS],
                    in1=t[:, 2 * BS:4 * BS],
                )
                nc.vector.tensor_add(
                    out=t[:, 0:BS], in0=t[:, 0:BS], in1=t[:, BS:2 * BS]
                )

            def project(dc, s):
                for blk in range(NDC):
                    nc.tensor.matmul(
                        po[:, blk * K:(blk + 1) * K],
                        lhsT=s[:, blk * P:(blk + 1) * P],
                        rhs=wt_sb[:, dc * K:(dc + 1) * K],
                        start=False,
                        stop=(dc == LD),
                    )

            fold(tL)  # dc7 quad 0 pre-folded early

            for dc in range(NDC - 1):
                t0, t1, t2 = quads[dc]
                nc.vector.tensor_add(out=t0[:], in0=t0[:], in1=t1[:])
                nc.vector.tensor_add(out=t0[:], in0=t0[:], in1=t2[:])
                fold(t0)
                project(dc, t0)

            for t in slices:
                nc.vector.tensor_add(
                    out=tL[:, 0:BS], in0=tL[:, 0:BS], in1=t[:]
                )
            project(LD, tL)

            ysb = opool.tile([P, NDC * K], F16)
            nc.scalar.activation(
                ysb[:], po[:], mybir.ActivationFunctionType.Tanh
            )
            nc.sync.dma_start(out=y.ap()[:], in_=ysb[:])

    nc.compile()
    _nc_cache = nc
    return nc


def _ensure_ntff_hook():
    """Register the axon NTFF profile hook if the image's antenv lacks it."""
    import sys
    import types

    try:
        from antenv.axon_hooks import get_axon_ntff_profile_hook  # noqa: F401
        return
    except ImportError:
        pass
    import antenv

    mod = types.ModuleType("antenv.axon_hooks")
    mod._hook = None

    def set_axon_ntff_profile_hook(h):
        mod._hook = h

    def get_axon_ntff_profile_hook():
        return mod._hook

    mod.set_axon_ntff_profile_hook = set_axon_ntff_profile_hook
    mod.get_axon_ntff_profile_hook = get_axon_ntff_profile_hook
    sys.modules["antenv.axon_hooks"] = mod
    antenv.axon_hooks = mod
    try:
        from trn_agent_boot.trn_boot import _ntff_profile_via_ctypes

        mod._hook = _ntff_profile_via_ctypes("/opt/axon/libaxon_pjrt.so")
    except Exception:
        mod._hook = None


def _run(inputs, trace=False, **kwargs):
    x = np.asarray(inputs["x"], dtype=np.float32)
    W = np.asarray(inputs["W"], dtype=np.float32)
    b = np.asarray(inputs["b"], dtype=np.float32)
    wtq = (W.T * np.float32(1.0 / L)).astype(np.float16)    # [1024(d), 64]
    wt = np.ascontiguousarray(
        wtq.reshape(NDC, P, K).transpose(1, 0, 2)
    ).reshape(P, NDC * K)
    bias = np.tile(b.astype(np.float16), NDC).reshape(1, NDC * K)
    in_maps = []
    for c in range(NCORES):
        xs = x[:, c * BS:(c + 1) * BS, :]            # [12, 1024(b), 1024(d)]
        xq = xs.reshape(NLQ, 4, BS, D).transpose(0, 3, 1, 2)
        xq = np.ascontiguousarray(xq, dtype=np.float16).reshape(NLQ, D, QW)
        in_maps.append({"x": xq, "wt": wt, "bias": bias})
    if trace:
        _ensure_ntff_hook()
        import concourse.bass_utils as bu

        bu.upload_artifacts = lambda tmpdir: "local://skipped"
    nc = _build()
    res = run_bass_kernel_spmd(
        nc, in_maps, core_ids=list(range(NCORES)), trace=trace, **kwargs
    )
    ys = []
    for r in res.results:
        yr = r["y"].astype(np.float32)
        yr = yr.reshape(P, NDC, K).transpose(1, 0, 2).reshape(BS, K)
        ys.append(np.ascontiguousarray(yr))
    return np.concatenate(ys, axis=0), res


def kernel(**inputs):
    y, _ = _run(inputs)
    return y
